# revision 87
# baseline (speedup 1.0000x reference)
"""Trainium2 Bass kernel for nn_MultiHeadAttention_32031866093611.

Sharding: pure data parallel - batch b -> NeuronCore b (B == n_cores == 8).
Weights replicated. No collectives. 226341 ns cost-model time vs 281994 ns
for the fp32r baseline; max rel err ~8.2e-3 vs fp64 reference.

Design:
  - q/k/v projections in fp8e4m3 with hi/lo error compensation
    (x = xhi + xlo, 32W = Whi + Wlo; q ~= xhi@Whi + xhi@Wlo + xlo@Whi, all
    three terms accumulated in one PSUM group) using DoubleRow perf mode:
    lhsT [K,2,M] / rhs [K,2,N] pack two 128-row contraction tiles per matmul
    at 0.5 cycles/row -> 24 matmuls/chunk at ~107 ns vs 16 at ~213 ns.
    The 1/32 rescale + bias fold into the PSUM->SBUF bias copy (DVE
    tensor_scalar with per-partition bias AP). hi/lo splits and W pretiling
    are host-side.
  - qT/kT stored f32r (scores matmuls are ldweights-free and keep the
    fp8 residual as the only q/k error); et/v/attn/Wo/masks bf16 (halves
    SBUF/DMA; DVE 2x_1p on the mask multiplies; exp writes bf16 directly).
  - scores PSUM is split into [128,512] half-tiles (pp pool bufs=4, one
    PSUM bank each) so the exp pipeline is 4 deep; PV accumulators and
    o-proj accumulators are also 1-bank half tiles (po pool bufs=4).
  - v tiles are [128, 16, 128] with ones in cols 64:127: the PV matmul
    replicates the softmax denominator into PSUM rows 64:128 for free
    (matmul cost is free-size only), so the norm is one partition-shifted
    DVE reciprocal + 4 strided multiplies into the head-interleaved attn
    layout - no gpsimd broadcast, no separate denominator pass.
  - schedule: one long (head, kc) stream with PV pops lagging scores by up
    to 14 half-tile slots and carried across head/pair boundaries; v chunks
    and next-chunk q/k projections and o-proj chunks are interleaved into
    the stream as PE filler for the ACT-bound exp cadence. o-proj(c-1) is
    only emitted after head 2c's PVs and norm are fully emitted (its PSUM
    slot aliases pso0(c) - ordering is correctness-critical).
  - masks: host-built multiplicative 0/1 bf16 masks on the exp output
    (exp of raw scores is finite in bf16; masked cols then multiply to 0).
"""

import numpy as np
import ml_dtypes

import concourse.bass as bass
import concourse.mybir as mybir
import concourse.tile as tile
from concourse import bacc
from concourse.bass_utils import run_bass_kernel_spmd

B, S, D, H = 8, 1024, 1024, 16
DK = D // H  # 64
P = 128
NCHUNK = S // P  # 8
NCORES = 8
F32 = mybir.dt.float32
F32R = mybir.dt.float32r
BF16 = mybir.dt.bfloat16
FP8 = mybir.dt.float8e4
EXP = mybir.ActivationFunctionType.Exp
COPY = mybir.ActivationFunctionType.Copy
MULT = mybir.AluOpType.mult
ADD = mybir.AluOpType.add
DR = mybir.MatmulPerfMode.DoubleRow
HALF = 512
WSC = 32.0  # host scales W by 32 so fp8 sees ~unit-variance values
MSK_OFF = [0]
for _kc in range(1, 8):
    MSK_OFF.append(MSK_OFF[-1] + S - (_kc - 1) * P)

_CACHED = {}


def build_nc(repeats=1):
    nc = bacc.Bacc("TRN2", target_bir_lowering=False, debug=False, num_devices=NCORES)

    xhi_d = nc.dram_tensor("xhi", [P, NCHUNK, S], FP8, kind="ExternalInput").ap()
    xlo_d = nc.dram_tensor("xlo", [P, NCHUNK, S], FP8, kind="ExternalInput").ap()
    wq8_d = {
        hl: nc.dram_tensor(f"wq8{hl}", [NCHUNK, P, 4, 2, P], FP8, kind="ExternalInput").ap()
        for hl in ("h", "l")
    }
    wk8_d = {
        hl: nc.dram_tensor(f"wk8{hl}", [NCHUNK, P, 4, 2, P], FP8, kind="ExternalInput").ap()
        for hl in ("h", "l")
    }
    wv8_d = {
        hl: nc.dram_tensor(f"wv8{hl}", [P, 4, 2, S], FP8, kind="ExternalInput").ap()
        for hl in ("h", "l")
    }
    wo_d = nc.dram_tensor("wo16", [P, NCHUNK, S], BF16, kind="ExternalInput").ap()
    bqk_d = nc.dram_tensor("bqk", [P, 2 * NCHUNK], F32, kind="ExternalInput").ap()
    bv_d = nc.dram_tensor("bv16", [P, D], BF16, kind="ExternalInput").ap()
    bo_d = nc.dram_tensor("bo32", [P, D], F32, kind="ExternalInput").ap()
    msk_d = nc.dram_tensor("msk16", [P, 4608], BF16, kind="ExternalInput").ap()
    out_d = nc.dram_tensor("out", [S, D], F32, kind="ExternalOutput").ap()

    with tile.TileContext(nc) as tc:
        with (
            tc.tile_pool(name="cst", bufs=1) as cstpool,
            tc.tile_pool(name="qk", bufs=2) as qkpool,
            tc.tile_pool(name="v", bufs=8) as vpool,
            tc.tile_pool(name="exp", bufs=16) as exppool,
            tc.tile_pool(name="rbc", bufs=2) as rbcpool,
            tc.tile_pool(name="osb", bufs=2) as osbpool,
            tc.tile_pool(name="big", bufs=2) as bigpool,
            tc.tile_pool(name="pp", bufs=4, space="PSUM") as pp,
            tc.tile_pool(name="po", bufs=4, space="PSUM") as po,
        ):
            for _rep in range(repeats):
                # ---- PE warm-up (no DMA dependency) ----
                warm = cstpool.tile([P, P], BF16, tag="warm")
                nc.vector.memzero(warm[:])
                wps = pp.tile([P, HALF], F32, tag="pp", name="warmup_ps")
                for wi in range(26):
                    nc.tensor.matmul(wps[:, 0:P], warm[:], warm[:], start=True, stop=True)

                # ---- DMAs, ordered for earliest dependency release ----
                xhi = cstpool.tile([P, NCHUNK, S], FP8, tag="xhi")
                bqk = cstpool.tile([P, 2 * NCHUNK], F32, tag="bqk")
                nc.sync.dma_start(bqk[:], bqk_d[:])
                nc.sync.dma_start(xhi[:, 0:4], xhi_d[:, 0:4])
                wq8 = {
                    hl: cstpool.tile([P, NCHUNK, 4, 2, P], FP8, tag=f"wq8{hl}", name=f"wq8{hl}")
                    for hl in ("h", "l")
                }
                wk8 = {
                    hl: cstpool.tile([P, NCHUNK, 4, 2, P], FP8, tag=f"wk8{hl}", name=f"wk8{hl}")
                    for hl in ("h", "l")
                }
                wv8 = {
                    hl: cstpool.tile([P, 4, 2, S], FP8, tag=f"wv8{hl}", name=f"wv8{hl}")
                    for hl in ("h", "l")
                }
                for hl in ("h", "l"):
                    nc.sync.dma_start(wq8[hl][:, 0], wq8_d[hl][0])
                nc.sync.dma_start(xhi[:, 4:8], xhi_d[:, 4:8])
                for hl in ("h", "l"):
                    nc.sync.dma_start(wk8[hl][:, 0], wk8_d[hl][0])
                xlo = cstpool.tile([P, NCHUNK, S], FP8, tag="xlo")
                nc.sync.dma_start(xlo[:, 0:4], xlo_d[:, 0:4])
                nc.sync.dma_start(xlo[:, 4:8], xlo_d[:, 4:8])
                nc.sync.dma_start(wv8["h"][:], wv8_d["h"][:])
                nc.sync.dma_start(wv8["l"][:], wv8_d["l"][:])
                bv16 = cstpool.tile([P, D], BF16, tag="bv16")
                nc.sync.dma_start(bv16[:], bv_d[:])
                msk = cstpool.tile([P, 4608], BF16, tag="msk")
                nc.sync.dma_start(msk[:], msk_d[:])
                wo16 = cstpool.tile([P, NCHUNK, S], BF16, tag="wo16")
                nc.sync.dma_start(wo16[:], wo_d[:])
                bo32 = cstpool.tile([P, D], F32, tag="bo32")
                nc.sync.dma_start(bo32[:], bo_d[:])

                def dma_wqk_cb(c):
                    for tl, dr in ((wq8, wq8_d), (wk8, wk8_d)):
                        for hl in ("h", "l"):
                            nc.sync.dma_start(tl[hl][:, c], dr[hl][c])

                dma_wqk_cb(1)

                # ---- fp8 DoubleRow projection helpers ----
                def proj_qk(c, w8, bcol, out_tag):
                  with nc.named_scope(f"pj_{out_tag}_{c}"):
                    o = qkpool.tile([P, S], F32R, tag=out_tag, name=f"{out_tag}_{c}")
                    for hf in range(2):
                        sl = slice(hf * HALF, (hf + 1) * HALF)
                        ps = pp.tile([P, HALF], F32, tag="pp", name=f"ps_{out_tag}_{c}_{hf}")
                        seq = [(w8["h"][:, c, j], xhi[:, 2 * j : 2 * j + 2, sl]) for j in range(4)]
                        seq += [(w8["l"][:, c, j], xhi[:, 2 * j : 2 * j + 2, sl]) for j in range(4)]
                        seq += [(w8["h"][:, c, j], xlo[:, 2 * j : 2 * j + 2, sl]) for j in range(4)]
                        for i, (l, r) in enumerate(seq):
                            nc.tensor.matmul(
                                ps[:], l, r,
                                start=(i == 0), stop=(i == len(seq) - 1), perf_mode=DR,
                            )
                        nc.vector.tensor_scalar(o[:, sl], ps[:], 1.0 / WSC, bqk[:, bcol + c : bcol + c + 1], MULT, ADD)
                    return o

                def proj_v(sc):
                  with nc.named_scope(f"pj_v_{sc}"):
                    ssl = slice(sc * P, (sc + 1) * P)
                    vt = vpool.tile([P, H, P], BF16, tag="v", name=f"v_{sc}")
                    # ones in cols 64:128 -> PV psum rows 64:128 hold the
                    # denominator replicated (free: matmul cost is free-size)
                    nc.gpsimd.memset(vt[:], 1.0)
                    for hf in range(2):
                        sl = slice(hf * HALF, (hf + 1) * HALF)
                        ps = pp.tile([P, HALF], F32, tag="pp", name=f"ps_v_{sc}_{hf}")
                        seq = [(xhi[:, 2 * j : 2 * j + 2, ssl], wv8["h"][:, j, :, sl]) for j in range(4)]
                        seq += [(xlo[:, 2 * j : 2 * j + 2, ssl], wv8["h"][:, j, :, sl]) for j in range(4)]
                        seq += [(xhi[:, 2 * j : 2 * j + 2, ssl], wv8["l"][:, j, :, sl]) for j in range(4)]
                        for i, (l, r) in enumerate(seq):
                            nc.tensor.matmul(
                                ps[:], l, r,
                                start=(i == 0), stop=(i == len(seq) - 1), perf_mode=DR,
                            )
                        nc.scalar.activation(
                            vt[:, hf * 8 : (hf + 1) * 8, 0:DK],
                            ps[:].rearrange("p (h d) -> p h d", h=8),
                            COPY, scale=1.0 / WSC,
                        )
                    nc.vector.tensor_add(
                        vt[:, :, 0:DK],
                        vt[:, :, 0:DK],
                        bv16[:].rearrange("p (h d) -> p h d", h=H),
                    )
                    return vt

                # ---- attention pieces ----
                qT, kT, vtiles = [None] * NCHUNK, [None] * NCHUNK, [None] * NCHUNK
                attn = [None, None]

                def scores_exp(h, kc):
                    nm = nc.named_scope(f"sc_{h}_{kc}")
                    nm.__enter__()
                    c, r = h // 2, (h % 2) * DK
                    lhs = kT[c][r : r + DK, kc * P : (kc + 1) * P]
                    et = exppool.tile([P, S], BF16, tag="et", name=f"et_{h}_{kc}")
                    off = MSK_OFF[kc]
                    for hf in range(2):
                        sl = slice(hf * HALF, (hf + 1) * HALF)
                        ps = pp.tile([P, HALF], F32, tag="pp", name=f"pss_{h}_{kc}_{hf}")
                        nc.tensor.matmul(ps[:], lhs, qT[c][r : r + DK, sl], start=True, stop=True)
                        nc.scalar.activation(et[:, sl], ps[:], EXP)
                        lo = max(kc * P, hf * HALF)
                        hi_ = (hf + 1) * HALF
                        if lo < hi_:
                            moff = off + (lo - kc * P)
                            nc.vector.tensor_mul(
                                et[:, lo:hi_], et[:, lo:hi_], msk[:, moff : moff + hi_ - lo]
                            )
                    nm.__exit__(None, None, None)
                    return et

                def emit_pv(h, kc, pso, et):
                  with nc.named_scope(f"pv_{h}_{kc}"):
                    for hf in range(2):
                        sl = slice(hf * HALF, (hf + 1) * HALF)
                        nc.tensor.matmul(
                            pso[hf][:, :], vtiles[kc][:, h, :], et[:, sl],
                            start=(kc == 0), stop=(kc == NCHUNK - 1),
                        )

                def emit_norm(h, pso, hf):
                  with nc.named_scope(f"norm_{h}_{hf}"):
                    ph = pso[hf]
                    rbc = rbcpool.tile([DK, HALF], F32, tag="rbc", name=f"rbc_{h}_{hf}")
                    nc.vector.reciprocal(rbc[:], ph[DK : 2 * DK, :])
                    # q in [hf*512, (hf+1)*512) -> u = q//16 in [hf*32, (hf+1)*32)
                    src = ph[0:DK, :].rearrange("d (u j) -> d j u", j=16)
                    rbs = rbc[:].rearrange("d (u j) -> d j u", j=16)
                    usl = slice(h * DK + hf * 32, h * DK + (hf + 1) * 32)
                    for g in range(2):
                        if attn[g] is None:
                            attn[g] = bigpool.tile([P, 4, S], BF16, tag="big", name=f"attnq_{g}")
                        for e in range(2):
                            jsl = slice(8 * g + e, 8 * (g + 1), 2)
                            nc.vector.tensor_mul(
                                attn[g][e * DK : (e + 1) * DK, :, usl],
                                src[:, jsl, :], rbs[:, jsl, :],
                            )

                osb_cur = [None]

                def emit_oproj_half(sc, hf):
                  with nc.named_scope(f"oproj_{sc}_{hf}"):
                    if hf == 0:
                        osb_cur[0] = osbpool.tile([P, S], F32, tag="osb", name=f"ot_{sc}")
                    ot = osb_cur[0]
                    sl = slice(hf * HALF, (hf + 1) * HALF)
                    ps = po.tile([P, HALF], F32, tag="po", name=f"psf_{sc}_{hf}")
                    for cc in range(NCHUNK):
                        nc.tensor.matmul(
                            ps[:],
                            attn[cc // 4][:, cc % 4, sc * P : (sc + 1) * P],
                            wo16[:, cc, sl],
                            start=(cc == 0), stop=(cc == NCHUNK - 1),
                        )
                    nc.vector.tensor_add(ot[:, sl], ps[:], bo32[:, sl])
                    nc.sync.dma_start(out_d[sc * P : (sc + 1) * P, sl], ot[:, sl])

                def emit_oproj(sc):
                    emit_oproj_half(sc, 0)
                    emit_oproj_half(sc, 1)

                # ---- pre-phase: qk chunk 0, hi terms of BOTH chunks first so
                # the PE stays busy while the xlo DMA lands; xlo terms after ----
                def proj_qk0_hi(w8, out_tag):
                    o = qkpool.tile([P, S], F32R, tag=out_tag, name=f"{out_tag}_0")
                    pss = []
                    for hf in range(2):
                        sl = slice(hf * HALF, (hf + 1) * HALF)
                        ps = pp.tile([P, HALF], F32, tag="pp", name=f"ps0_{out_tag}_{hf}")
                        seq = [(w8["h"][:, 0, j], xhi[:, 2 * j : 2 * j + 2, sl]) for j in range(4)]
                        seq += [(w8["l"][:, 0, j], xhi[:, 2 * j : 2 * j + 2, sl]) for j in range(4)]
                        for i, (l, r) in enumerate(seq):
                            nc.tensor.matmul(ps[:], l, r, start=(i == 0), stop=False, perf_mode=DR)
                        pss.append(ps)
                    return o, pss

                def proj_qk0_lo(w8, bcol, o, pss):
                    for hf in range(2):
                        sl = slice(hf * HALF, (hf + 1) * HALF)
                        seq = [(w8["h"][:, 0, j], xlo[:, 2 * j : 2 * j + 2, sl]) for j in range(4)]
                        for i, (l, r) in enumerate(seq):
                            nc.tensor.matmul(
                                pss[hf][:], l, r, start=False, stop=(i == len(seq) - 1), perf_mode=DR
                            )
                        nc.vector.tensor_scalar(o[:, sl], pss[hf][:], 1.0 / WSC, bqk[:, bcol : bcol + 1], MULT, ADD)
                    return o

                _q0, _q0ps = proj_qk0_hi(wq8, "qT")
                _k0, _k0ps = proj_qk0_hi(wk8, "kT")
                qT[0] = proj_qk0_lo(wq8, 0, _q0, _q0ps)
                kT[0] = proj_qk0_lo(wk8, NCHUNK, _k0, _k0ps)
                vtiles[0] = proj_v(0)


                # ---- pair loop (pend carried across pair boundaries) ----
                from collections import deque

                pend = deque()

                def pop_pv():
                    ph, pkc, ppso, pet = pend.popleft()
                    emit_pv(ph, pkc, ppso, pet)
                    if pkc == NCHUNK - 1:
                        emit_norm(ph, ppso, 0)
                        emit_norm(ph, ppso, 1)

                for c in range(NCHUNK):
                    h0, h1 = 2 * c, 2 * c + 1
                    if c < NCHUNK - 2:
                        dma_wqk_cb(c + 2)
                    pso0 = [po.tile([P, HALF], F32, tag="po", name=f"pso_{h0}_{hf}") for hf in range(2)]
                    for kc in range(NCHUNK):
                        et = scores_exp(h0, kc)
                        if c == 0 and kc % 2 == 1:
                            vtiles[1 + kc // 2] = proj_v(1 + kc // 2)
                        # eagerly drain PVs of previous heads; keep lag 4 for own
                        thr = 6 if c == NCHUNK - 1 else 14
                        if len(pend) >= thr or (
                            pend and pend[0][0] < h0 and (c < NCHUNK - 1 or len(pend) > 2)
                        ):
                            pop_pv()
                        pend.append((h0, kc, pso0, et))

                    if c == NCHUNK - 1:
                        while pend and pend[0][0] < h0:
                            pop_pv()
                        emit_oproj(NCHUNK - 2)
                    pso1 = [po.tile([P, HALF], F32, tag="po", name=f"pso_{h1}_{hf}") for hf in range(2)]
                    for kc in range(NCHUNK):
                        et = scores_exp(h1, kc)
                        if c == 0 and kc % 2 == 1 and kc < 6:
                            vtiles[5 + kc // 2] = proj_v(5 + kc // 2)
                        if len(pend) >= 14 or (pend and pend[0][0] < h0):
                            pop_pv()
                        if c == NCHUNK - 1 and pend:
                            pop_pv()
                        pend.append((h1, kc, pso1, et))
                    if c < NCHUNK - 1:
                        qT[c + 1] = proj_qk(c + 1, wq8, 0, "qT")
                        pop_pv()
                        pop_pv()
                        kT[c + 1] = proj_qk(c + 1, wk8, NCHUNK, "kT")
                        if c >= 1:
                            # oproj(c-1) reuses pso0(c)'s PSUM slot: h0's PVs and
                            # norm must be fully emitted first (race otherwise)
                            while pend and pend[0][0] <= h0:
                                pop_pv()
                            emit_oproj(c - 1)
                    else:
                        while pend:
                            pop_pv()
                emit_oproj(NCHUNK - 1)

    nc.compile()
    return nc


def _host_masks(prefix_b: int):
    """Multiplicative 0/1 mask (bf16) applied to exp output."""
    i = np.arange(P)[:, None]
    segs = []
    for kc in range(NCHUNK):
        q = np.arange(kc * P, S)[None, :]
        k = kc * P + i
        allowed = (q < prefix_b) | (k >= q)
        segs.append(allowed.astype(np.float32))
    return np.concatenate(segs, axis=1).astype(ml_dtypes.bfloat16)


def _split8(a):
    hi = a.astype(ml_dtypes.float8_e4m3fn)
    lo = (a - hi.astype(np.float32)).astype(ml_dtypes.float8_e4m3fn)
    return hi, lo


def _pack_wqk(w):
    """[8cb, 128k, 4j, 2t, 128m] from W32 [(2j+t)*128+k, cb*128+m]."""
    a = (w * WSC).reshape(4, 2, P, NCHUNK, P).transpose(3, 2, 0, 1, 4)
    return _split8(np.ascontiguousarray(a))


def _pack_wv(w):
    """[128k, 4j, 2t, 1024n] from Wv32 [(2j+t)*128+k, n]."""
    a = (w * WSC).reshape(4, 2, P, S).transpose(2, 0, 1, 3)
    return _split8(np.ascontiguousarray(a))


def kernel(x, prefix, Wq, bq, Wk, bk, Wv, bv, Wo, bo, _trace=False):
    x = np.asarray(x, dtype=np.float32)
    prefix = np.asarray(prefix)
    Wq, Wk, Wv, Wo = (np.asarray(w, np.float32) for w in (Wq, Wk, Wv, Wo))
    bqk = np.stack(
        [np.asarray(bq, np.float32).reshape(NCHUNK, P), np.asarray(bk, np.float32).reshape(NCHUNK, P)],
        axis=0,
    ).reshape(2 * NCHUNK, P).T.copy()  # [128, 16]: cols 0-7 bq chunks, 8-15 bk

    wq8h, wq8l = _pack_wqk(Wq)
    wk8h, wk8l = _pack_wqk(Wk)
    wv8h, wv8l = _pack_wv(Wv)
    wo16 = np.ascontiguousarray(
        Wo.reshape(NCHUNK, P, S).transpose(1, 0, 2)
    ).astype(ml_dtypes.bfloat16)
    bv16 = np.broadcast_to(np.asarray(bv, np.float32).reshape(1, D), (P, D)).astype(ml_dtypes.bfloat16)
    bo32 = np.broadcast_to(np.asarray(bo, np.float32).reshape(1, D), (P, D)).astype(np.float32).copy()

    if "nc" not in _CACHED:
        _CACHED["nc"] = build_nc()
    nc = _CACHED["nc"]

    in_maps = []
    for b in range(B):
        xt = np.ascontiguousarray(x[b].T)  # [D, S]
        xts = np.ascontiguousarray(xt.reshape(NCHUNK, P, S).transpose(1, 0, 2))  # [128, 8, 1024]
        xhi, xlo = _split8(xts)
        mask16 = _host_masks(int(prefix[b]))
        in_maps.append(
            {
                "xhi": xhi, "xlo": xlo,
                "wq8h": wq8h, "wq8l": wq8l,
                "wk8h": wk8h, "wk8l": wk8l,
                "wv8h": wv8h, "wv8l": wv8l,
                "wo16": wo16, "bqk": bqk, "bv16": bv16, "bo32": bo32,
                "msk16": mask16,
            }
        )

    res = run_bass_kernel_spmd(nc, in_maps, core_ids=list(range(NCORES)), trace=_trace)
    out = np.stack([res.results[b]["out"] for b in range(B)], axis=0)
    if _trace:
        return out, res
    return out
